# revision 1
# baseline (speedup 1.0000x reference)
"""Trainium2 distributed kernel: 4-layer attention encoder (B=4, D=1024, H=16, N=1024).

Sharding: (batch, sequence-half) across 8 NeuronCores — core r owns batch
b = r//2 and sequence half r%2 (512 columns). All conv1x1 projections and
the MLP are per-column -> fully local. Per layer each core computes its
K / V^T shard and AllGathers it with its batch peer only (2-rank groups),
then runs attention for its 512 query columns of its batch.

Host-side preprocessing (exact, fp32):
  - channel permutation to head-major so each head's 64 channels are contiguous
  - 1/sqrt(DK) folded into Wq/bq
  - bk dropped (constant-per-row shift is softmax invariant)
  - bv folded into the merge bias (softmax rows sum to 1): bm_eff = bm + Wm @ bv
  - BatchNorm (eval) + p1 bias folded to per-channel scale/bias applied in the
    Relu activation: h = relu(s1 * p1_raw + b1)
  - streamed lhsT weights packed per output-tile so every weight DMA is one
    contiguous block

Compute dtype: bf16 matmul inputs, fp32 PSUM accumulation; the residual
stream stays fp32 end-to-end.
"""

import numpy as np
import ml_dtypes

import concourse.bass as bass
import concourse.mybir as mybir
import concourse.tile as tile
from concourse import bacc
from concourse.bass_utils import run_bass_kernel_spmd

L, D, H, B, N = 4, 1024, 16, 4, 1024
DK = D // H          # 64
R = 8                # cores
NS = N // 2          # 512 per-core sequence columns (one batch, half sequence)
DT = D // 128        # 8 d-tiles
NT = NS // 128       # 4 n-tiles per core
BF = mybir.dt.bfloat16
F32 = mybir.dt.float32
BFNP = ml_dtypes.bfloat16

# head-major channel permutation: perm[h*64+dk] = dk*16+h
PERM = np.array([dk * H + h for h in range(H) for dk in range(DK)])


def _wtile_stream(w_t):
    """(C, M) weight -> (M//128, 128, C//128*128): arr[mt, p, ct*128+mo] =
    w_t[ct*128+p, mt*128+mo]. Each [mt] block is one contiguous lhsT tile."""
    c, m = w_t.shape
    a = w_t.reshape(c // 128, 128, m // 128, 128)      # (ct, p, mt, mo)
    a = a.transpose(2, 1, 0, 3)                        # (mt, p, ct, mo)
    return np.ascontiguousarray(a.reshape(m // 128, 128, -1)).astype(BFNP)


def _wtile_res(w_t):
    """(C, M) weight -> (128, C//128*M) [p, ct*M + m] for resident rhs/lhsT use."""
    c, m = w_t.shape
    return np.ascontiguousarray(
        w_t.reshape(c // 128, 128, m).transpose(1, 0, 2).reshape(128, -1)
    ).astype(BFNP)


def _btile(b_vec):
    """(C,) bias -> (128, C//128) [p, ct]."""
    c = b_vec.shape[0]
    return np.ascontiguousarray(b_vec.reshape(c // 128, 128).T).astype(np.float32)


def prepare_host_inputs(inputs):
    """Preprocess full weights once; returns dict of shard-independent arrays."""
    Wq, bq = inputs["Wq"], inputs["bq"]
    Wk = inputs["Wk"]
    Wv, bv = inputs["Wv"], inputs["bv"]
    Wm, bm = inputs["Wm"], inputs["bm"]
    Wp1, bp1 = inputs["Wp1"], inputs["bp1"]
    g, beta = inputs["bn_gamma"], inputs["bn_beta"]
    mu, var = inputs["bn_mean"], inputs["bn_var"]
    Wp2 = inputs["Wp2"]

    out = {k: [] for k in ("wq", "wk", "wv", "wm", "wp1", "wp2", "bq", "bm", "s1", "b1")}
    for l in range(L):
        out["wq"].append(_wtile_stream((Wq[l][PERM] / 8.0).T))
        out["wk"].append(_wtile_stream(Wk[l][PERM].T))
        out["wv"].append(_wtile_res(Wv[l][PERM].T))
        out["wm"].append(_wtile_res(Wm[l][:, PERM].T))
        out["wp1"].append(_wtile_stream(Wp1[l].T))
        out["wp2"].append(_wtile_stream(Wp2[l].T))
        out["bq"].append(_btile(bq[l][PERM] / 8.0))
        bm_eff = bm[l] + Wm[l] @ bv[l]
        out["bm"].append(_btile(bm_eff))
        s1 = g[l] / np.sqrt(var[l] + 1e-5)
        b1 = beta[l] + s1 * (bp1[l] - mu[l])
        out["s1"].append(_btile(s1))
        out["b1"].append(_btile(b1))
    res = {k: np.stack(v) for k, v in out.items()}
    # biases: (L, 128, C) -> (128, L*C) so the device DMA is a plain copy
    for k in ("bq", "bm", "s1", "b1"):
        res[k] = np.ascontiguousarray(res[k].transpose(1, 0, 2).reshape(128, -1))
    res["ident"] = np.eye(128, dtype=BFNP)
    return res


def shard_x(motion_feats, r):
    """(B, D, N) -> core r's (128, DT*NS) fp32 tile layout [p, ct*NS + n]."""
    b, half = r // 2, r % 2
    m = motion_feats[b, :, half * NS : (half + 1) * NS]    # (D, NS)
    m = m.reshape(DT, 128, NS).transpose(1, 0, 2)          # (p, ct, n)
    return np.ascontiguousarray(m.reshape(128, DT * NS)).astype(np.float32)


def unshard_out(res_list):
    """8 x (128, DT*NS) -> (B, D, N)."""
    out = np.empty((B, D, N), dtype=np.float32)
    for r, arr in enumerate(res_list):
        b, half = r // 2, r % 2
        m = arr.reshape(128, DT, NS).transpose(1, 0, 2)    # (ct, p, n)
        out[b, :, half * NS : (half + 1) * NS] = m.reshape(D, NS)
    return out


def build_nc():
    nc = bacc.Bacc("TRN2", target_bir_lowering=False, debug=False, num_devices=R)

    x_in = nc.dram_tensor("x_in", [128, DT * NS], F32, kind="ExternalInput")
    wq = nc.dram_tensor("wq", [L, DT, 128, D], BF, kind="ExternalInput")
    wk = nc.dram_tensor("wk", [L, DT, 128, D], BF, kind="ExternalInput")
    wv = nc.dram_tensor("wv", [L, 128, DT * D], BF, kind="ExternalInput")
    wm = nc.dram_tensor("wm", [L, 128, DT * D], BF, kind="ExternalInput")
    wp1 = nc.dram_tensor("wp1", [L, 16, 128, 2048], BF, kind="ExternalInput")
    wp2 = nc.dram_tensor("wp2", [L, DT, 128, 2048], BF, kind="ExternalInput")
    bq_d = nc.dram_tensor("bq", [128, L * 8], F32, kind="ExternalInput")
    bm_d = nc.dram_tensor("bm", [128, L * 8], F32, kind="ExternalInput")
    s1_d = nc.dram_tensor("s1", [128, L * 16], F32, kind="ExternalInput")
    b1_d = nc.dram_tensor("b1", [128, L * 16], F32, kind="ExternalInput")
    out_e = nc.dram_tensor("out", [128, DT * NS], F32, kind="ExternalOutput")

    ADD = mybir.AluOpType.add
    AF = mybir.ActivationFunctionType
    GROUPS = [[0, 1], [2, 3], [4, 5], [6, 7]]

    with tile.TileContext(nc) as tc:
        with (
            tc.tile_pool(name="const", bufs=1) as const,
            tc.tile_pool(name="acts", bufs=1) as acts,
            tc.tile_pool(name="wres", bufs=1) as wres,
            tc.tile_pool(name="wstr", bufs=3) as wstr,
            tc.tile_pool(name="kv", bufs=8) as kvp,
            tc.tile_pool(name="attn_t", bufs=2) as attp,
            tc.tile_pool(name="wts", bufs=4) as wtsp,
            tc.tile_pool(name="small", bufs=6) as smallp,
            tc.tile_pool(name="pp", bufs=3, space="PSUM") as ppp,
            tc.tile_pool(name="sc", bufs=2, space="PSUM") as scp,
            tc.tile_pool(name="at", bufs=1, space="PSUM") as atp,
            tc.tile_pool(name="dram", bufs=2, space="DRAM") as dramp,
        ):
            bq_sb = const.tile([128, L * 8], F32)
            nc.sync.dma_start(bq_sb[:], bq_d[:, :])
            bm_sb = const.tile([128, L * 8], F32)
            nc.sync.dma_start(bm_sb[:], bm_d[:, :])
            s1_sb = const.tile([128, L * 16], F32)
            nc.sync.dma_start(s1_sb[:], s1_d[:, :])
            b1_sb = const.tile([128, L * 16], F32)
            nc.sync.dma_start(b1_sb[:], b1_d[:, :])

            x_sb = acts.tile([128, DT * NS], F32)
            nc.sync.dma_start(x_sb[:], x_in[:, :])
            x_bf = acts.tile([128, DT * NS], BF)
            q_bf = acts.tile([128, DT * NS], BF)
            attn_bf = acts.tile([128, DT * NS], BF)
            mg_bf = acts.tile([128, DT * NS], BF)
            h1_bf = acts.tile([128, 16 * NS], BF)
            k_sh = acts.tile([128, DT * NS], BF)
            v_sh = acts.tile([128, NT * D], BF)

            def stream_w(src, l, mt, tag):
                """One contiguous lhsT m-tile: all contraction chunks for mt."""
                t = wstr.tile([128, src.shape[3]], BF, tag=tag)
                nc.sync.dma_start(t[:], src[l, mt, :, :])
                return t

            for l in range(L):
                # cast residual stream to bf16 once per layer
                nc.vector.tensor_copy(x_bf[:], x_sb[:])

                # ---- K projection (feeds the collective first) ----
                for mt in range(DT):
                    w_t = stream_w(wk, l, mt, "wqk")
                    ps = ppp.tile([128, NS], F32, tag="pp")
                    for ct in range(DT):
                        nc.tensor.matmul(
                            ps[:],
                            w_t[:, ct * 128 : (ct + 1) * 128],
                            x_bf[:, ct * NS : (ct + 1) * NS],
                            start=(ct == 0),
                            stop=(ct == DT - 1),
                        )
                    nc.vector.tensor_copy(k_sh[:, mt * NS : (mt + 1) * NS], ps[:])

                # ---- V^T projection: out[n, d], n-tiles of 128 ----
                wv_sb = wres.tile([128, DT * D], BF, tag="wv")
                nc.sync.dma_start(wv_sb[:], wv[l, :, :])
                for nt in range(NT):
                    for dh in range(2):
                        ps = ppp.tile([128, NS], F32, tag="pp")
                        for ct in range(DT):
                            nc.tensor.matmul(
                                ps[:],
                                x_bf[:, ct * NS + nt * 128 : ct * NS + (nt + 1) * 128],
                                wv_sb[:, ct * D + dh * 512 : ct * D + (dh + 1) * 512],
                                start=(ct == 0),
                                stop=(ct == DT - 1),
                            )
                        nc.vector.tensor_copy(
                            v_sh[:, nt * D + dh * 512 : nt * D + (dh + 1) * 512], ps[:]
                        )

                # ---- AllGather V^T (overlaps q-proj and the score matmuls) ----
                cv_i = dramp.tile([128, NT * D], BF, tag="cvi")
                nc.sync.dma_start(cv_i[:, :], v_sh[:])
                cv_o = dramp.tile([2 * 128, NT * D], BF, tag="cvo")
                nc.gpsimd.collective_compute(
                    "AllGather",
                    mybir.AluOpType.bypass,
                    replica_groups=GROUPS,
                    ins=[cv_i[:].opt()],
                    outs=[cv_o[:].opt()],
                )

                # ---- AllGather K with the batch peer (2-rank); V follows ----
                ck_i = dramp.tile([128, DT * NS], BF, tag="cki")
                nc.sync.dma_start(ck_i[:, :], k_sh[:])
                ck_o = dramp.tile([2 * 128, DT * NS], BF, tag="cko")
                nc.gpsimd.collective_compute(
                    "AllGather",
                    mybir.AluOpType.bypass,
                    replica_groups=GROUPS,
                    ins=[ck_i[:].opt()],
                    outs=[ck_o[:].opt()],
                )
                for mt in range(DT):
                    w_t = stream_w(wq, l, mt, "wqk")
                    ps = ppp.tile([128, NS], F32, tag="pp")
                    for ct in range(DT):
                        nc.tensor.matmul(
                            ps[:],
                            w_t[:, ct * 128 : (ct + 1) * 128],
                            x_bf[:, ct * NS : (ct + 1) * NS],
                            start=(ct == 0),
                            stop=(ct == DT - 1),
                        )
                    nc.vector.tensor_scalar_add(
                        q_bf[:, mt * NS : (mt + 1) * NS],
                        ps[:],
                        bq_sb[:, l * 8 + mt : l * 8 + mt + 1],
                    )

                # prefetch merge weights while attention runs
                wm_sb = wres.tile([128, DT * D], BF, tag="wm")
                nc.sync.dma_start(wm_sb[:], wm[l, :, :])

                # gathered views:
                #   ck_o rows r2*128 + p: K shard [p, mt*NS+ns] (d = mt*128+p, n = r2*NS+ns)
                #   cv_o rows r2*128 + p: V^T shard [p, nt*D+d] (m = r2*NS + nt*128 + p)
                ko = ck_o[:].rearrange("(r p) (mt ns) -> r p mt ns", r=2, mt=DT)
                vo = cv_o[:].rearrange("(r p) (nt d) -> r p nt d", r=2, nt=NT)

                # K tiles: kt[t][p, m] (d = t*128+p, m = 0..1023)
                kts = []
                for t in range(DT):
                    kt = kvp.tile([128, N], BF, tag="kt")
                    nc.sync.dma_start(
                        kt[:].rearrange("p (r ns) -> p r ns", r=2),
                        ko[:, :, t, :].rearrange("r p ns -> p r ns"),
                    )
                    kts.append(kt)
                # V^T tiles: vt[mc][p, d] (m = mc*128+p)
                vts = []
                for mc in range(DT):
                    vt = kvp.tile([128, D], BF, tag="vt")
                    nc.sync.dma_start(vt[:], vo[mc // NT, :, mc % NT, :])
                    vts.append(vt)

                # ---- attention: 8 head-pairs x 4 query n-tiles ----
                for t in range(DT):
                    for nt in range(NT):
                        wts = []
                        for hi in range(2):
                            h = 2 * t + hi
                            half = hi * 64
                            sc = scp.tile([128, N], F32, tag="sc")
                            lhsT_q = q_bf[
                                half : half + 64,
                                t * NS + nt * 128 : t * NS + (nt + 1) * 128,
                            ]
                            for j in range(2):
                                nc.tensor.matmul(
                                    sc[:, j * 512 : (j + 1) * 512],
                                    lhsT_q,
                                    kts[t][half : half + 64, j * 512 : (j + 1) * 512],
                                    start=True,
                                    stop=True,
                                )
                            w_raw = attp.tile([128, N], BF, tag="wraw")
                            ssum = smallp.tile([128, 1], F32, tag="ssum")
                            nc.scalar.activation(
                                w_raw[:], sc[:], AF.Exp, accum_out=ssum[:]
                            )
                            rinv = smallp.tile([128, 1], F32, tag="rinv")
                            nc.vector.reciprocal(rinv[:], ssum[:])
                            w_bf = attp.tile([128, N], BF, tag="wbf")
                            nc.vector.tensor_scalar_mul(w_bf[:], w_raw[:], rinv[:])
                            # chunked transpose via the DMA xbar:
                            # wt[p, r*128+n] = w_bf[n, r*128+p]
                            wt_sb = wtsp.tile([128, N], BF, tag="wtsb")
                            nc.sync.dma_start(
                                wt_sb[:].rearrange("p (r n) -> p r n", r=DT),
                                w_bf[:],
                                transpose=True,
                            )
                            wts.append(wt_sb)

                        at = atp.tile([128, 128], F32, tag="at")
                        for hi in range(2):
                            for r in range(DT):
                                nc.tensor.matmul(
                                    at[hi * 64 : (hi + 1) * 64, :],
                                    vts[r][:, (2 * t + hi) * 64 : (2 * t + hi + 1) * 64],
                                    wts[hi][:, r * 128 : (r + 1) * 128],
                                    start=(r == 0),
                                    stop=(r == DT - 1),
                                )
                        nc.vector.tensor_copy(
                            attn_bf[:, t * NS + nt * 128 : t * NS + (nt + 1) * 128],
                            at[:],
                        )

                # ---- merge ----
                for mt in range(DT):
                    ps = ppp.tile([128, NS], F32, tag="pp")
                    for ct in range(DT):
                        nc.tensor.matmul(
                            ps[:],
                            wm_sb[:, ct * D + mt * 128 : ct * D + (mt + 1) * 128],
                            attn_bf[:, ct * NS : (ct + 1) * NS],
                            start=(ct == 0),
                            stop=(ct == DT - 1),
                        )
                    nc.vector.tensor_scalar_add(
                        mg_bf[:, mt * NS : (mt + 1) * NS],
                        ps[:],
                        bm_sb[:, l * 8 + mt : l * 8 + mt + 1],
                    )

                # ---- p1 + BN + relu (contraction: 8 merged chunks + 8 x chunks) ----
                for mt in range(16):
                    w_t = stream_w(wp1, l, mt, "wp1")
                    ps = ppp.tile([128, NS], F32, tag="pp")
                    for ct in range(16):
                        rhs = (
                            mg_bf[:, ct * NS : (ct + 1) * NS]
                            if ct < 8
                            else x_bf[:, (ct - 8) * NS : (ct - 7) * NS]
                        )
                        nc.tensor.matmul(
                            ps[:],
                            w_t[:, ct * 128 : (ct + 1) * 128],
                            rhs,
                            start=(ct == 0),
                            stop=(ct == 15),
                        )
                    nc.scalar.activation(
                        h1_bf[:, mt * NS : (mt + 1) * NS],
                        ps[:],
                        AF.Relu,
                        bias=b1_sb[:, l * 16 + mt : l * 16 + mt + 1],
                        scale=s1_sb[:, l * 16 + mt : l * 16 + mt + 1],
                    )

                # ---- p2 + residual ----
                for ot in range(DT):
                    w_t = stream_w(wp2, l, ot, "wp2")
                    ps = ppp.tile([128, NS], F32, tag="pp")
                    for ct in range(16):
                        nc.tensor.matmul(
                            ps[:],
                            w_t[:, ct * 128 : (ct + 1) * 128],
                            h1_bf[:, ct * NS : (ct + 1) * NS],
                            start=(ct == 0),
                            stop=(ct == 15),
                        )
                    nc.vector.tensor_tensor(
                        x_sb[:, ot * NS : (ot + 1) * NS],
                        x_sb[:, ot * NS : (ot + 1) * NS],
                        ps[:],
                        op=ADD,
                    )

            nc.sync.dma_start(out_e[:, :], x_sb[:])

    nc.finalize()
    return nc


_CACHED = {}


def kernel(**inputs):
    inputs = {k: np.asarray(v) for k, v in inputs.items()}
    host = prepare_host_inputs(inputs)

    if "nc" not in _CACHED:
        _CACHED["nc"] = build_nc()
    nc = _CACHED["nc"]

    in_maps = []
    for r in range(R):
        m = {
            "x_in": shard_x(inputs["motion_feats"], r),
            "wq": host["wq"], "wk": host["wk"], "wv": host["wv"], "wm": host["wm"],
            "wp1": host["wp1"], "wp2": host["wp2"],
            "bq": host["bq"], "bm": host["bm"], "s1": host["s1"], "b1": host["b1"],
        }
        in_maps.append(m)

    res = run_bass_kernel_spmd(nc, in_maps, core_ids=list(range(R)))
    return unshard_out([res.results[r]["out"] for r in range(R)])



# revision 3
# speedup vs baseline: 2.8381x; 2.8381x over previous
"""Trainium2 distributed kernel v2: 4-layer attention encoder (B=4, D=1024, H=16, N=1024).

Sharding: (batch, sequence-half) across 8 NeuronCores — core r owns batch
b = r//2 and sequence half r%2 (512 columns). Per layer each core computes its
K / V^T shard and AllGathers it with its batch peer (2-rank groups).

v2 changes vs v1:
  - fp8(e4m3) DoubleRow matmuls (256-deep contraction per instruction) for the
    Q/K/V projections, attn@V, merge, and the merged-half of p1. Scores, the
    x-half of p1, and p2 stay bf16 (numerics), residual stream fp32.
  - transposed-scores attention: scores^T[keys, q] computed directly, exp'd to
    fp8 on the Scalar engine; attn@V consumes w^T as the moving operand so the
    per-head softmax transpose of v1 (the xbar bottleneck) is gone entirely.
  - softmax row-sums via an all-ones DoubleRow lhsT whose M=128 output is the
    row-sum broadcast across all partitions; normalization is a per-partition
    reciprocal + multiply fused into the PSUM->SBUF drain.
  - p1's x-half is precomputed into SBUF right after the projections, which
    also covers the K/V AllGather latency before attention starts.

Host-side preprocessing (exact, fp32):
  - head-major channel permutation; 1/sqrt(DK) folded into the q drain
  - bk dropped (softmax-invariant); bv folded into the merge bias
  - BatchNorm folded into the p1 relu drain (scale/bias)
  - per-tensor power-of-2 fp8 scales for weights, folded into drain scalars
"""

import numpy as np
import ml_dtypes

import concourse.bass as bass
import concourse.mybir as mybir
import concourse.tile as tile
from concourse import bacc
from concourse.bass_utils import run_bass_kernel_spmd

L, D, H, B, N = 4, 1024, 16, 4, 1024
DK = D // H          # 64
R = 8                # cores
NS = N // 2          # 512 per-core sequence columns
DT = D // 128        # 8 d-tiles
NT = NS // 128       # 4 n-tiles per core
BF = mybir.dt.bfloat16
F32 = mybir.dt.float32
FP8 = mybir.dt.float8e4
BFNP = ml_dtypes.bfloat16
E4 = ml_dtypes.float8_e4m3

KX8 = 4.0    # x -> fp8 scale 2^4
KV8 = 4.0    # v -> fp8 scale 2^4
KM8 = 4.0    # merged -> fp8 scale 2^4

# head-major channel permutation: perm[h*64+dk] = dk*16+h
PERM = np.array([dk * H + h for h in range(H) for dk in range(DK)])


def _kexp(w, target=112.0):
    """Power-of-2 exponent so |w|*2^k <= target."""
    m = float(np.abs(w).max())
    if m == 0.0:
        return 0.0
    return float(np.floor(np.log2(target / m)))


def _wtile_stream(w_t, np_dtype):
    """(C, M) weight -> (M//128, 128, C) [mt, p, ct*128+mo] = w_t[ct*128+p, mt*128+mo].
    Works for both plain bf16 chunks (ct-major) and fp8 DoubleRow pairs
    (pair cp occupies cols cp*256..cp*256+255 with j-stride 128)."""
    c, m = w_t.shape
    a = w_t.reshape(c // 128, 128, m // 128, 128).transpose(2, 1, 0, 3)
    return np.ascontiguousarray(a.reshape(m // 128, 128, -1)).astype(np_dtype)


def _btile(b_vec):
    """(C,) bias -> (128, C//128) [p, ct]."""
    c = b_vec.shape[0]
    return np.ascontiguousarray(b_vec.reshape(c // 128, 128).T).astype(np.float32)


def prepare_host_inputs(inputs):
    Wq, bq = inputs["Wq"], inputs["bq"]
    Wk = inputs["Wk"]
    Wv, bv = inputs["Wv"], inputs["bv"]
    Wm, bm = inputs["Wm"], inputs["bm"]
    Wp1, bp1 = inputs["Wp1"], inputs["bp1"]
    g, beta = inputs["bn_gamma"], inputs["bn_beta"]
    mu, var = inputs["bn_mean"], inputs["bn_var"]
    Wp2 = inputs["Wp2"]

    out = {k: [] for k in ("wq", "wk", "wv", "wm", "wp1m", "wp1x", "wp2",
                           "bq", "mgb", "s1", "b1")}
    scal = {k: [] for k in ("sq", "sk", "sv", "sm")}
    for l in range(L):
        Wq_p = Wq[l][PERM]          # (D out head-major, D in)
        Wk_p = Wk[l][PERM]
        Wv_p = Wv[l][PERM]
        kq = _kexp(Wq_p); kk = _kexp(Wk_p); kv = _kexp(Wv_p)
        out["wq"].append(_wtile_stream((Wq_p * 2.0 ** kq).T, E4))
        out["wk"].append(_wtile_stream((Wk_p * 2.0 ** kk).T, E4))
        # wv resident layout [128, 8192]: [p, ct*1024 + d] = WvT[ct*128+p, d]
        wvt = (Wv_p * 2.0 ** kv).T   # (C in, D out)
        wv_r = wvt.reshape(DT, 128, D).transpose(1, 0, 2).reshape(128, DT * D)
        out["wv"].append(np.ascontiguousarray(wv_r).astype(E4))

        Wm_eff = Wm[l][:, PERM]     # input side head-major
        bm_eff = bm[l] + Wm[l] @ bv[l]
        km = _kexp(Wm_eff)
        out["wm"].append(_wtile_stream((Wm_eff * 2.0 ** km).T, E4))
        out["mgb"].append(_btile(bm_eff * 2.0 ** KM8))

        Wp1m = Wp1[l][:, :D]
        Wp1x = Wp1[l][:, D:]
        kp1 = _kexp(Wp1m)
        out["wp1m"].append(_wtile_stream((Wp1m * 2.0 ** (kp1 - KM8)).T, E4))
        out["wp1x"].append(_wtile_stream((Wp1x * 2.0 ** kp1).T, BFNP))
        out["wp2"].append(_wtile_stream(Wp2[l].T, BFNP))

        out["bq"].append(_btile(bq[l][PERM] / 8.0))
        s1 = g[l] / np.sqrt(var[l] + 1e-5)
        b1 = beta[l] + s1 * (bp1[l] - mu[l])
        out["s1"].append(_btile(s1 * 2.0 ** (-kp1)))
        out["b1"].append(_btile(b1))

        scal["sq"].append(2.0 ** (-kq - KX8) / 8.0)
        scal["sk"].append(2.0 ** (-kk - KX8))
        scal["sv"].append(2.0 ** (KV8 - kv - KX8))
        scal["sm"].append(2.0 ** (KM8 - km - KV8))

    res = {k: np.stack(v) for k, v in out.items()}
    for k in ("bq", "mgb"):
        res[k] = np.ascontiguousarray(res[k].transpose(1, 0, 2).reshape(128, -1))
    for k in ("s1", "b1"):
        res[k] = np.ascontiguousarray(res[k].transpose(1, 0, 2).reshape(128, -1))
    res["ident"] = np.eye(128, dtype=BFNP)
    res["_scal"] = {k: tuple(v) for k, v in scal.items()}
    return res


def shard_x(motion_feats, r):
    b, half = r // 2, r % 2
    m = motion_feats[b, :, half * NS : (half + 1) * NS]
    m = m.reshape(DT, 128, NS).transpose(1, 0, 2)
    return np.ascontiguousarray(m.reshape(128, DT * NS)).astype(np.float32)


def unshard_out(res_list):
    out = np.empty((B, D, N), dtype=np.float32)
    for r, arr in enumerate(res_list):
        b, half = r // 2, r % 2
        m = arr.reshape(128, DT, NS).transpose(1, 0, 2)
        out[b, :, half * NS : (half + 1) * NS] = m.reshape(D, NS)
    return out


def build_nc(scal):
    sq_l, sk_l, sv_l, sm_l = scal["sq"], scal["sk"], scal["sv"], scal["sm"]

    nc = bacc.Bacc("TRN2", target_bir_lowering=False, debug=False, num_devices=R)

    x_in = nc.dram_tensor("x_in", [128, DT * NS], F32, kind="ExternalInput")
    wq_d = nc.dram_tensor("wq", [L, DT, 128, D], FP8, kind="ExternalInput")
    wk_d = nc.dram_tensor("wk", [L, DT, 128, D], FP8, kind="ExternalInput")
    wv_d = nc.dram_tensor("wv", [L, 128, DT * D], FP8, kind="ExternalInput")
    wm_d = nc.dram_tensor("wm", [L, DT, 128, D], FP8, kind="ExternalInput")
    wp1m_d = nc.dram_tensor("wp1m", [L, 16, 128, D], FP8, kind="ExternalInput")
    wp1x_d = nc.dram_tensor("wp1x", [L, 16, 128, D], BF, kind="ExternalInput")
    wp2_d = nc.dram_tensor("wp2", [L, DT, 128, 2 * D], BF, kind="ExternalInput")
    bq_d = nc.dram_tensor("bq", [128, L * 8], F32, kind="ExternalInput")
    mgb_d = nc.dram_tensor("mgb", [128, L * 8], F32, kind="ExternalInput")
    s1_d = nc.dram_tensor("s1", [128, L * 16], F32, kind="ExternalInput")
    b1_d = nc.dram_tensor("b1", [128, L * 16], F32, kind="ExternalInput")
    out_e = nc.dram_tensor("out", [128, DT * NS], F32, kind="ExternalOutput")

    ADD = mybir.AluOpType.add
    MUL = mybir.AluOpType.mult
    AF = mybir.ActivationFunctionType
    DR = mybir.MatmulPerfMode.DoubleRow
    GROUPS = [[0, 1], [2, 3], [4, 5], [6, 7]]

    with tile.TileContext(nc) as tc:
        with (
            tc.tile_pool(name="const", bufs=1) as const,
            tc.tile_pool(name="acts", bufs=1) as acts,
            tc.tile_pool(name="wres", bufs=1) as wres,
            tc.tile_pool(name="wstr", bufs=3) as wstr,
            tc.tile_pool(name="wt8", bufs=3) as wt8p,
            tc.tile_pool(name="scl", bufs=3) as sclp,
            tc.tile_pool(name="pp", bufs=2, space="PSUM") as ppp,
            tc.tile_pool(name="sc", bufs=2, space="PSUM") as scp,
            tc.tile_pool(name="av", bufs=1, space="PSUM") as avp,
            tc.tile_pool(name="dram", bufs=2, space="DRAM") as dramp,
        ):
            bq_sb = const.tile([128, L * 8], F32)
            nc.sync.dma_start(bq_sb[:], bq_d[:, :])
            mgb_sb = const.tile([128, L * 8], F32)
            nc.sync.dma_start(mgb_sb[:], mgb_d[:, :])
            s1_sb = const.tile([128, L * 16], F32)
            nc.sync.dma_start(s1_sb[:], s1_d[:, :])
            b1_sb = const.tile([128, L * 16], F32)
            nc.sync.dma_start(b1_sb[:], b1_d[:, :])
            ones8 = const.tile([128, 256], FP8)
            nc.vector.memset(ones8[:], 1.0)

            x_sb = acts.tile([128, DT * NS], F32)
            nc.sync.dma_start(x_sb[:], x_in[:, :])
            x_bf = acts.tile([128, DT * NS], BF)
            x_f8 = acts.tile([128, DT * NS], FP8)
            q_bf = acts.tile([128, DT * NS], BF)
            k_sh = acts.tile([128, DT * NS], BF)
            v_sh = acts.tile([128, NT * D], FP8)
            kts = acts.tile([128, DT * N], BF)       # gathered K: [p, t*1024 + m]
            v_all = acts.tile([128, 2 * NT * D], FP8)  # gathered V^T: [p, c*1024 + d]
            attn_f8 = acts.tile([128, DT * NS], FP8)
            mg_f8 = acts.tile([128, DT * NS], FP8)
            h1x = acts.tile([128, 16 * NS], F32)
            h1_bf = acts.tile([128, 16 * NS], BF)

            def xpair(cp):
                return x_f8[:, 2 * cp * NS : (2 * cp + 2) * NS].rearrange(
                    "p (j n) -> p j n", j=2
                )

            def wpair(t, cp):
                return t[:, cp * 256 : (cp + 1) * 256].rearrange(
                    "p (j m) -> p j m", j=2
                )

            for l in range(L):
                # ---- casts ----
                nc.vector.tensor_scalar_mul(x_f8[:], x_sb[:], 2.0 ** KX8)
                nc.vector.tensor_copy(x_bf[:], x_sb[:])

                # ---- K projection (fp8 DR), drains on Scalar ----
                for mt in range(DT):
                    w_t = wstr.tile([128, D], FP8, tag="wk")
                    nc.sync.dma_start(w_t[:], wk_d[l, mt, :, :])
                    ps = ppp.tile([128, NS], F32, tag="pp")
                    for cp in range(4):
                        nc.tensor.matmul(
                            ps[:], wpair(w_t, cp), xpair(cp),
                            start=(cp == 0), stop=(cp == 3), perf_mode=DR,
                        )
                    nc.scalar.activation(
                        k_sh[:, mt * NS : (mt + 1) * NS], ps[:], AF.Copy,
                        scale=sk_l[l],
                    )
                ck_i = dramp.tile([128, DT * NS], BF, tag="cki")
                nc.sync.dma_start(ck_i[:, :], k_sh[:])
                ck_o = dramp.tile([2 * 128, DT * NS], BF, tag="cko")
                nc.gpsimd.collective_compute(
                    "AllGather", mybir.AluOpType.bypass, replica_groups=GROUPS,
                    ins=[ck_i[:].opt()], outs=[ck_o[:].opt()],
                )

                # ---- V^T projection (fp8 DR) ----
                wv_sb = wres.tile([128, DT * D], FP8, tag="wv")
                nc.sync.dma_start(wv_sb[:], wv_d[l, :, :])
                for nt in range(NT):
                    for dh in range(2):
                        ps = ppp.tile([128, NS], F32, tag="pp")
                        for cp in range(4):
                            lhsT = xpair(cp)[:, :, nt * 128 : (nt + 1) * 128]
                            rhs = wv_sb[
                                :, 2 * cp * D : (2 * cp + 2) * D
                            ].rearrange("p (j d) -> p j d", j=2)[
                                :, :, dh * NS : (dh + 1) * NS
                            ]
                            nc.tensor.matmul(
                                ps[:], lhsT, rhs,
                                start=(cp == 0), stop=(cp == 3), perf_mode=DR,
                            )
                        nc.scalar.activation(
                            v_sh[:, nt * D + dh * NS : nt * D + (dh + 1) * NS],
                            ps[:], AF.Copy, scale=sv_l[l],
                        )
                cv_i = dramp.tile([128, NT * D], FP8, tag="cvi")
                nc.sync.dma_start(cv_i[:, :], v_sh[:])
                cv_o = dramp.tile([2 * 128, NT * D], FP8, tag="cvo")
                nc.gpsimd.collective_compute(
                    "AllGather", mybir.AluOpType.bypass, replica_groups=GROUPS,
                    ins=[cv_i[:].opt()], outs=[cv_o[:].opt()],
                )

                # ---- Q projection (fp8 DR), drain on DVE with bias ----
                for mt in range(DT):
                    w_t = wstr.tile([128, D], FP8, tag="wq")
                    nc.sync.dma_start(w_t[:], wq_d[l, mt, :, :])
                    ps = ppp.tile([128, NS], F32, tag="pp")
                    for cp in range(4):
                        nc.tensor.matmul(
                            ps[:], wpair(w_t, cp), xpair(cp),
                            start=(cp == 0), stop=(cp == 3), perf_mode=DR,
                        )
                    nc.vector.tensor_scalar(
                        q_bf[:, mt * NS : (mt + 1) * NS], ps[:],
                        sq_l[l], bq_sb[:, l * 8 + mt : l * 8 + mt + 1],
                        op0=MUL, op1=ADD,
                    )

                # ---- p1 x-half (bf16), covers the AllGather latency ----
                for mt in range(16):
                    w_t = wstr.tile([128, D], BF, tag="wp1x")
                    nc.sync.dma_start(w_t[:], wp1x_d[l, mt, :, :])
                    ps = ppp.tile([128, NS], F32, tag="pp")
                    for ct in range(DT):
                        nc.tensor.matmul(
                            ps[:],
                            w_t[:, ct * 128 : (ct + 1) * 128],
                            x_bf[:, ct * NS : (ct + 1) * NS],
                            start=(ct == 0), stop=(ct == DT - 1),
                        )
                    nc.vector.tensor_copy(h1x[:, mt * NS : (mt + 1) * NS], ps[:])

                # ---- gathered K/V into SBUF ----
                ko = ck_o[:].rearrange("(r p) (t m) -> r p t m", r=2, t=DT)
                nc.sync.dma_start(
                    kts[:].rearrange("p (t r m) -> p t r m", t=DT, r=2),
                    ko.rearrange("r p t m -> p t r m"),
                )
                vo = cv_o[:].rearrange("(r p) (nt d) -> r p nt d", r=2, nt=NT)
                nc.sync.dma_start(
                    v_all[:].rearrange("p (r nt d) -> p r nt d", r=2, nt=NT),
                    vo.rearrange("r p nt d -> p r nt d"),
                )

                # ---- attention: one-head-lookahead pipeline ----
                def sc_head(h):
                    t, half = h // 2, (h % 2) * 64
                    wt_t = wt8p.tile([128, 8 * NS], FP8, tag="wt")
                    for cp in range(4):
                        scps = scp.tile([128, 2 * NS], F32, tag="sc")
                        for j in range(2):
                            c = 2 * cp + j
                            nc.tensor.matmul(
                                scps[:, j * NS : (j + 1) * NS],
                                kts[half : half + 64, t * N + c * 128 : t * N + (c + 1) * 128],
                                q_bf[half : half + 64, t * NS : (t + 1) * NS],
                                start=True, stop=True,
                            )
                        nc.scalar.activation(
                            wt_t[:, cp * 2 * NS : (cp + 1) * 2 * NS], scps[:], AF.Exp
                        )
                    return wt_t

                def cons_head(h, wt_t):
                    t, half = h // 2, h % 2
                    rsb = avp.tile([128, NS], F32, tag="rsb")
                    for cp in range(4):
                        nc.tensor.matmul(
                            rsb[:],
                            ones8[:].rearrange("p (j m) -> p j m", j=2),
                            wt_t[:, cp * 2 * NS : (cp + 1) * 2 * NS].rearrange(
                                "p (j n) -> p j n", j=2
                            ),
                            start=(cp == 0), stop=(cp == 3), perf_mode=DR,
                        )
                    at = avp.tile([128, NS], F32, tag="at")
                    for cp in range(4):
                        lhsT = v_all[
                            :, 2 * cp * D : (2 * cp + 2) * D
                        ].rearrange("p (j d) -> p j d", j=2)[
                            :, :, t * 128 : (t + 1) * 128
                        ]
                        nc.tensor.matmul(
                            at[:], lhsT,
                            wt_t[:, cp * 2 * NS : (cp + 1) * 2 * NS].rearrange(
                                "p (j n) -> p j n", j=2
                            ),
                            start=(cp == 0), stop=(cp == 3), perf_mode=DR,
                        )
                    # normalize + cast the correct half into attn_f8
                    sl = slice(half * 64, half * 64 + 64)
                    rcp = sclp.tile([128, NS], F32, tag="rcp")
                    nc.vector.reciprocal(rcp[sl, :], rsb[sl, :])
                    nc.vector.tensor_tensor(
                        attn_f8[sl, t * NS : (t + 1) * NS],
                        at[sl, :], rcp[sl, :], op=MUL,
                    )

                prev = sc_head(0)
                for h in range(1, 16):
                    cur = sc_head(h)
                    cons_head(h - 1, prev)
                    prev = cur
                cons_head(15, prev)

                # ---- merge (fp8 DR) ----
                for mt in range(DT):
                    w_t = wstr.tile([128, D], FP8, tag="wm")
                    nc.sync.dma_start(w_t[:], wm_d[l, mt, :, :])
                    ps = ppp.tile([128, NS], F32, tag="pp")
                    for cp in range(4):
                        nc.tensor.matmul(
                            ps[:], wpair(w_t, cp),
                            attn_f8[:, 2 * cp * NS : (2 * cp + 2) * NS].rearrange(
                                "p (j n) -> p j n", j=2
                            ),
                            start=(cp == 0), stop=(cp == 3), perf_mode=DR,
                        )
                    nc.vector.tensor_scalar(
                        mg_f8[:, mt * NS : (mt + 1) * NS], ps[:],
                        sm_l[l], mgb_sb[:, l * 8 + mt : l * 8 + mt + 1],
                        op0=MUL, op1=ADD,
                    )

                # ---- p1 merged-half (fp8 DR) + h1x + BN/relu ----
                for mt in range(16):
                    w_t = wstr.tile([128, D], FP8, tag="wp1m")
                    nc.sync.dma_start(w_t[:], wp1m_d[l, mt, :, :])
                    ps = ppp.tile([128, NS], F32, tag="pp")
                    for cp in range(4):
                        nc.tensor.matmul(
                            ps[:], wpair(w_t, cp),
                            mg_f8[:, 2 * cp * NS : (2 * cp + 2) * NS].rearrange(
                                "p (j n) -> p j n", j=2
                            ),
                            start=(cp == 0), stop=(cp == 3), perf_mode=DR,
                        )
                    nc.vector.tensor_tensor(
                        ps[:], ps[:], h1x[:, mt * NS : (mt + 1) * NS], op=ADD
                    )
                    nc.scalar.activation(
                        h1_bf[:, mt * NS : (mt + 1) * NS], ps[:], AF.Relu,
                        bias=b1_sb[:, l * 16 + mt : l * 16 + mt + 1],
                        scale=s1_sb[:, l * 16 + mt : l * 16 + mt + 1],
                    )

                # ---- p2 (bf16) + residual ----
                for ot in range(DT):
                    w_t = wstr.tile([128, 2 * D], BF, tag="wp2")
                    nc.sync.dma_start(w_t[:], wp2_d[l, ot, :, :])
                    ps = ppp.tile([128, NS], F32, tag="pp")
                    for ct in range(16):
                        nc.tensor.matmul(
                            ps[:],
                            w_t[:, ct * 128 : (ct + 1) * 128],
                            h1_bf[:, ct * NS : (ct + 1) * NS],
                            start=(ct == 0), stop=(ct == 15),
                        )
                    nc.vector.tensor_tensor(
                        x_sb[:, ot * NS : (ot + 1) * NS],
                        x_sb[:, ot * NS : (ot + 1) * NS],
                        ps[:], op=ADD,
                    )

            nc.sync.dma_start(out_e[:, :], x_sb[:])

    nc.finalize()
    return nc


_CACHED = {}


def kernel(**inputs):
    inputs = {k: np.asarray(v) for k, v in inputs.items()}
    host = prepare_host_inputs(inputs)
    scal = host.pop("_scal")

    key = tuple(sorted(scal.items()))
    if _CACHED.get("key") != key:
        _CACHED["nc"] = build_nc(scal)
        _CACHED["key"] = key
    nc = _CACHED["nc"]

    in_maps = build_in_maps(inputs, host)
    res = run_bass_kernel_spmd(nc, in_maps, core_ids=list(range(R)))
    return unshard_out([res.results[r]["out"] for r in range(R)])


def build_in_maps(inputs, host):
    in_maps = []
    for r in range(R):
        m = {
            "x_in": shard_x(inputs["motion_feats"], r),
            "wq": host["wq"], "wk": host["wk"], "wv": host["wv"], "wm": host["wm"],
            "wp1m": host["wp1m"], "wp1x": host["wp1x"], "wp2": host["wp2"],
            "bq": host["bq"], "mgb": host["mgb"], "s1": host["s1"], "b1": host["b1"],
        }
        in_maps.append(m)
    return in_maps
